# revision 1
# baseline (speedup 1.0000x reference)
"""AutoCompleteDecoderModel (LSTM enc-dec + CE loss) on 8 Trainium2 cores.

Strategy (hardcoded for B=256, S=512, H=512, V=128):
 - Data-parallel: 8 cores x 32 samples.
 - Per core, per time step the recurrent matmul is computed transposed:
   gates.T [2048, 32] as 16 M-tiles of [128, 32] packed into ONE PSUM bank
   [128, 512] (col 32*m+j = gate-dim 128*m+p of sample j).  lhsT = W_hh.T
   tile [128,128] (stationary, bf16, FWL), rhs = packed hT [128, 32*k+j]
   column slices (bf16).
 - One-hot input projection: host precomputes one-hot streams [128, S*32]
   (bf16), fed as the first K-tile of each accumulation group; LSTM biases
   are folded into W_ih columns (each step consumes exactly one one-hot).
 - sigmoid(x) = 0.5*(1+tanh(x/2)) so only Tanh+Exp are used -> single ACT
   table set ("exp_and_others").  The 0.5 factors are folded by keeping the
   state hT_scaled = 2*h and halving W_hh / proj_W on the host.
 - Gate order permuted to [g, i, f, o] so the activation chain starts as
   soon as the first PSUM column-slices complete.
 - Decoder: logits.T [128, 32] = 0.5*proj_W @ hT via 4 k-MMs; exp(logits+b)
   (ACT bias) and logits*onehot(tgt) are stacked [128, 64] and reduced over
   the partition dim with a ones-matmul -> [1, 64] per step (sumexp|tgtdot).
   Host does: nll = ln(sumexp) - (tgtdot + proj_b[tgt]), masked mean, sum.
"""

import os
import sys

import numpy as np

if "/opt/trn_rl_repo" not in sys.path:
    sys.path.insert(0, "/opt/trn_rl_repo")

B, S, H, V = 256, 512, 512, 128
NCORES = 8
BS = B // NCORES  # 32 samples per core
U = 8  # steps per hw-loop iteration

_CACHE = {}


def _prep_weights(W_ih, W_hh, b_ih, b_hh):
    """Fold biases into W_ih, halve W_hh (hT=2h convention), permute gates
    i,f,g,o -> g,i,f,o, and tile for the kernel layouts."""
    import ml_dtypes

    # gate m-tile order [g|i|f|o]: g,i land in PSUM bank A (done after half
    # the k-MMs -> a1 computes early), f,o in bank B.  Tile serializes PSUM
    # reads per BANK, so the split is what lets the chain overlap the MMs.
    perm = np.concatenate([
        np.arange(1024, 1536),  # g
        np.arange(0, 512),      # i
        np.arange(512, 1024),   # f
        np.arange(1536, 2048),  # o
    ])
    Wi = (np.asarray(W_ih, np.float32) + (np.asarray(b_ih, np.float32)
          + np.asarray(b_hh, np.float32))[:, None])[perm]  # [2048, 128]
    Wh = 0.5 * np.asarray(W_hh, np.float32)[perm]          # [2048, 512]
    # input proj: lhsT tiles = W_ih'.T [V=128, 2048]
    wih_t = np.ascontiguousarray(Wi.T).astype(ml_dtypes.bfloat16)
    # recurrent: tiles [k, m] = W_hh'.T[128k:128k+128, 128m:128m+128]
    # packed [128, 64*128] with col (k*16+m)*128 + c
    Wt = np.ascontiguousarray(Wh.T)  # [512, 2048]
    whh_t = (Wt.reshape(4, 128, 16, 128).transpose(1, 0, 2, 3)
             .reshape(128, 8192).astype(ml_dtypes.bfloat16))
    return np.ascontiguousarray(wih_t), np.ascontiguousarray(whh_t)


def _onehot_stream(idx):
    """idx [BS, S] int -> [128, S*32] bf16, col t*32+j = (idx[j,t]==v)."""
    import ml_dtypes
    oh = (np.arange(V, dtype=np.int32)[:, None, None]
          == np.asarray(idx, np.int32).T[None, :, :])  # [V, S, BS]
    return np.ascontiguousarray(oh.reshape(V, -1).astype(ml_dtypes.bfloat16))


def _build_module(n_steps, unrolled=False):
    """Build + compile the Bass/Tile module (same program on all cores).
    unrolled=True replaces the hw For_i loops with static python loops
    (for cost-model simulation of small n_steps)."""
    import concourse.bacc as bacc
    import concourse.bass as bass
    import concourse.mybir as mybir
    import concourse.tile as tile

    f32 = mybir.dt.float32
    bf16 = mybir.dt.bfloat16
    AF = mybir.ActivationFunctionType
    OP = mybir.AluOpType
    PE = mybir.EngineType.PE

    n_iters = n_steps // U

    nc = bacc.Bacc("TRN2", target_bir_lowering=False, debug=False,
                   num_devices=NCORES)

    # ---- DRAM I/O ----
    d_enc_whh = nc.dram_tensor("enc_whh", [128, 8192], bf16, kind="ExternalInput").ap()
    d_enc_wih = nc.dram_tensor("enc_wih", [128, 2048], bf16, kind="ExternalInput").ap()
    d_dec_whh = nc.dram_tensor("dec_whh", [128, 8192], bf16, kind="ExternalInput").ap()
    d_dec_wih = nc.dram_tensor("dec_wih", [128, 2048], bf16, kind="ExternalInput").ap()
    d_projt = nc.dram_tensor("projt", [128, 512], bf16, kind="ExternalInput").ap()
    d_projb = nc.dram_tensor("projb", [128, 1], f32, kind="ExternalInput").ap()
    d_enc_oh = nc.dram_tensor("enc_oh", [128, n_steps * BS], bf16, kind="ExternalInput").ap()
    d_dec_oh = nc.dram_tensor("dec_oh", [128, n_steps * BS], bf16, kind="ExternalInput").ap()
    d_res = nc.dram_tensor("res", [1, n_steps * 64], f32, kind="ExternalOutput").ap()

    with tile.TileContext(nc) as tc:
        with (
            tc.tile_pool(name="const", bufs=1) as const_pool,
            tc.tile_pool(name="oh", bufs=2) as oh_pool,
            tc.tile_pool(name="gates", bufs=2, space="PSUM") as gates_pool,
            tc.tile_pool(name="gatesB", bufs=2, space="PSUM") as gatesB_pool,
            tc.tile_pool(name="logits", bufs=2, space="PSUM") as logits_pool,
            tc.tile_pool(name="resps", bufs=2, space="PSUM") as resps_pool,
            tc.tile_pool(name="work", bufs=3) as work_pool,
            tc.tile_pool(name="stack", bufs=2) as stack_pool,
            tc.tile_pool(name="acc", bufs=2) as acc_pool,
        ):
            # persistent weights + state
            w_enc_hh = const_pool.tile([128, 8192], bf16, tag="wehh")
            w_enc_ih = const_pool.tile([128, 2048], bf16, tag="weih")
            w_dec_hh = const_pool.tile([128, 8192], bf16, tag="wdhh")
            w_dec_ih = const_pool.tile([128, 2048], bf16, tag="wdih")
            w_projt = const_pool.tile([128, 512], bf16, tag="wpt")
            w_projb = const_pool.tile([128, 1], f32, tag="wpb")
            ones_col = const_pool.tile([128, 1], f32, tag="ones")
            hT = const_pool.tile([128, 128], bf16, tag="hT")
            # state sst = 2*c  (tanh(c) = tanh(0.5*sst) via ACT pre-scale)
            sst = const_pool.tile([128, 128], f32, tag="sst")

            nc.sync.dma_start(w_enc_hh[:], d_enc_whh)
            nc.sync.dma_start(w_enc_ih[:], d_enc_wih)
            nc.sync.dma_start(w_dec_hh[:], d_dec_whh)
            nc.sync.dma_start(w_dec_ih[:], d_dec_wih)
            nc.sync.dma_start(w_projt[:], d_projt)
            nc.sync.dma_start(w_projb[:], d_projb)
            nc.vector.memset(ones_col[:], 1.0)
            nc.vector.memset(hT[:], 0.0)
            nc.vector.memset(sst[:], 0.0)

            abl = int(os.environ.get("LSTM_ABLATE", "0"))

            def _pslice(pair, m):
                psA, psB = pair
                t = psA if m < 8 else psB
                mm = m % 8
                return t[:, mm * 32:(mm + 1) * 32]

            def inproj(w_ih, xt):
                """Input projection (+folded bias) for one step: K-tile 0 of
                each PSUM accumulation group.  No dependence on hT, so these
                matmuls fill the PE while the previous step's activation
                chain runs."""
                psA = gates_pool.tile([128, 256], f32, tag="psA")
                psB = gatesB_pool.tile([128, 256], f32, tag="psB")
                pair = (psA, psB)
                for m in range(16):
                    nc.tensor.matmul(_pslice(pair, m),
                                     w_ih[:, m * 128:(m + 1) * 128],
                                     xt, start=True, stop=False)
                return pair

            def k_mms(w_hh, pair):
                # recurrent K-tiles, m-major so PSUM slices finish in order
                for m in range(16):
                    for k in range(4):
                        nc.tensor.matmul(
                            _pslice(pair, m),
                            w_hh[:, (k * 16 + m) * 128:(k * 16 + m + 1) * 128],
                            hT[:, k * 32:(k + 1) * 32],
                            start=False, stop=(k == 3))

            def chain(pair):
                """Gate activations + state update.
                bank A cols: [g 0:128 | i 128:256]; bank B: [f 0:128 | o 128:256]
                sigmoid(x) = 0.5*(1+tanh(x/2)) via tanh + STT."""
                if abl == 3:
                    return
                psA, psB = pair
                gt = work_pool.tile([128, 128], f32, tag="gt")
                nc.scalar.activation(gt[:], psA[:, 0:128], AF.Tanh, scale=1.0)
                ti = work_pool.tile([128, 128], f32, tag="ti")
                nc.scalar.activation(ti[:], psA[:, 128:256], AF.Tanh, scale=0.5)
                a1 = work_pool.tile([128, 128], f32, tag="a1")
                nc.vector.scalar_tensor_tensor(a1[:], ti[:], 1.0, gt[:],
                                               OP.add, OP.mult)
                tf = work_pool.tile([128, 128], f32, tag="tf")
                nc.scalar.activation(tf[:], psB[:, 0:128], AF.Tanh, scale=0.5)
                a2 = work_pool.tile([128, 128], f32, tag="a2")
                nc.vector.scalar_tensor_tensor(a2[:], tf[:], 1.0, sst[:],
                                               OP.add, OP.mult)
                # sst_new = 0.5*(1+t_f)*sst + (1+t_i)*g~  (= 2*c_new)
                nc.vector.scalar_tensor_tensor(sst[:], a2[:], 0.5, a1[:],
                                               OP.mult, OP.add)
                tc2 = work_pool.tile([128, 128], f32, tag="tc2")
                nc.scalar.activation(tc2[:], sst[:], AF.Tanh, scale=0.5)
                to = work_pool.tile([128, 128], f32, tag="to")
                nc.scalar.activation(to[:], psB[:, 128:256], AF.Tanh, scale=0.5)
                if abl == 4:
                    hscr = work_pool.tile([128, 128], bf16, tag="hscr")
                    nc.vector.scalar_tensor_tensor(hscr[:], to[:], 1.0, tc2[:],
                                                   OP.add, OP.mult)
                else:
                    nc.vector.scalar_tensor_tensor(hT[:], to[:], 1.0, tc2[:],
                                                   OP.add, OP.mult)

            def dec_tail_mm(u):
                """Projection matmuls for step u's logits (reads updated hT)."""
                ps_l = logits_pool.tile([128, 32], f32, tag="psl")
                for k in range(4):
                    nc.tensor.matmul(ps_l[:, :],
                                     w_projt[:, k * 128:(k + 1) * 128],
                                     hT[:, k * 32:(k + 1) * 32],
                                     start=(k == 0), stop=(k == 3))
                return ps_l

            def dec_tail_reduce(ps_l, xt, accum, u):
                """exp+tgtdot stack, ones-matmul, copy into accum slice."""
                st = stack_pool.tile([128, 64], f32, tag="st")
                nc.scalar.activation(st[:, 0:32], ps_l[:, :], AF.Exp,
                                     bias=w_projb[:, 0:1], scale=1.0)
                nc.vector.tensor_mul(st[:, 32:64], ps_l[:, :], xt)
                rp = resps_pool.tile([1, 64], f32, tag="rp")
                nc.tensor.matmul(rp[:, :], ones_col[:, 0:1], st[:, :],
                                 start=True, stop=True)
                nc.scalar.activation(accum[:, u * 64:(u + 1) * 64], rp[:, :],
                                     AF.Copy)

            def enc_body(i):
                oh = oh_pool.tile([128, U * BS], bf16, tag="oh")
                nc.sync.dma_start(oh[:], d_enc_oh[:, bass.ts(i, U * BS)])
                # inproj(u+1) emitted between k_mms(u) and chain(u): PE
                # runs it while the chain produces hT(u).
                ps = inproj(w_enc_ih, oh[:, 0:BS])
                for u in range(U):
                    k_mms(w_enc_hh, ps)
                    if u + 1 < U:
                        ps_n = inproj(w_enc_ih, oh[:, (u + 1) * BS:(u + 2) * BS])
                    chain(ps)
                    ps = ps_n

            def dec_body(i):
                oh = oh_pool.tile([128, U * BS], bf16, tag="oh")
                nc.sync.dma_start(oh[:], d_dec_oh[:, bass.ts(i, U * BS)])
                accum = acc_pool.tile([1, U * 64], f32, tag="accum")
                ps = inproj(w_dec_ih, oh[:, 0:BS])
                pend_mm = None   # (u, xt) whose proj matmuls are due
                pend_red = None  # (ps_l, xt, accum, u) awaiting reduce
                for u in range(U):
                    xt = oh[:, u * BS:(u + 1) * BS]
                    k_mms(w_dec_hh, ps)
                    if u + 1 < U:
                        ps_n = inproj(w_dec_ih, oh[:, (u + 1) * BS:(u + 2) * BS])
                    if pend_mm is not None and abl != 5:
                        pu, pxt = pend_mm
                        if pend_red is not None:
                            dec_tail_reduce(*pend_red)
                        pend_red = (dec_tail_mm(pu), pxt, accum, pu)
                    pend_mm = (u, xt)
                    chain(ps)
                    ps = ps_n
                pu, pxt = pend_mm
                if pend_red is not None:
                    dec_tail_reduce(*pend_red)
                dec_tail_reduce(dec_tail_mm(pu), pxt, accum, pu)
                nc.sync.dma_start(d_res[:, bass.ts(i, U * 64)], accum[:])

            if unrolled:
                for i in range(n_iters):
                    enc_body(i)
                for i in range(n_iters):
                    dec_body(i)
            else:
                with tc.For_i(0, n_iters, 1, hint_engines=(PE,), name="enc") as i:
                    enc_body(i)
                with tc.For_i(0, n_iters, 1, hint_engines=(PE,), name="dec") as i:
                    dec_body(i)

    nc.compile()
    return nc


def _run(inputs, n_steps=S, trace=False):
    from concourse import bass_utils

    key = n_steps
    if key not in _CACHE:
        _CACHE[key] = _build_module(n_steps)
    nc = _CACHE[key]

    enc_wih, enc_whh = _prep_weights(inputs["enc_W_ih"], inputs["enc_W_hh"],
                                     inputs["enc_b_ih"], inputs["enc_b_hh"])
    dec_wih, dec_whh = _prep_weights(inputs["dec_W_ih"], inputs["dec_W_hh"],
                                     inputs["dec_b_ih"], inputs["dec_b_hh"])
    import ml_dtypes
    projW = 0.5 * np.asarray(inputs["proj_W"], np.float32)  # [128, 512]
    projt = (np.ascontiguousarray(projW.T).reshape(4, 128, 128)
             .transpose(1, 0, 2).reshape(128, 512).astype(ml_dtypes.bfloat16))
    projb = np.ascontiguousarray(
        np.asarray(inputs["proj_b"], np.float32).reshape(128, 1))

    C_idx = np.asarray(inputs["C_idx"])[:, :n_steps]
    E = np.asarray(inputs["E"])
    Etgt = E[:, :n_steps]

    in_maps = []
    for c in range(NCORES):
        sl = slice(c * BS, (c + 1) * BS)
        in_maps.append({
            "enc_whh": enc_whh, "enc_wih": enc_wih,
            "dec_whh": dec_whh, "dec_wih": dec_wih,
            "projt": np.ascontiguousarray(projt), "projb": projb,
            "enc_oh": _onehot_stream(C_idx[sl]),
            "dec_oh": _onehot_stream(Etgt[sl]),
        })

    res = bass_utils.run_bass_kernel_spmd(
        nc, in_maps, core_ids=list(range(NCORES)), trace=trace,
        trace_cores=[0] if trace else None)

    # ---- host-side loss assembly (float64) ----
    proj_b = np.asarray(inputs["proj_b"], np.float64)
    nll = np.empty((B, n_steps), np.float64)
    for c in range(NCORES):
        r = np.asarray(res.results[c]["res"], np.float64).reshape(n_steps, 64)
        sumexp = r[:, 0:32]          # [S, 32]
        tgtdot = r[:, 32:64]
        tgt = Etgt[c * BS:(c + 1) * BS]          # [32, S]
        bias_t = proj_b[tgt]                      # [32, S]
        nll[c * BS:(c + 1) * BS] = (np.log(sumexp).T
                                    - (tgtdot.T + bias_t))
    mask = (Etgt != 0).astype(np.float64)         # [B, S]
    num = (nll * mask).sum(axis=0)
    cnt = mask.sum(axis=0)
    step_loss = np.where(cnt > 0, num / np.maximum(cnt, 1.0), 0.0)
    total = np.float32(step_loss.sum())
    return total, res


def kernel(**inputs) -> np.ndarray:
    total, _ = _run(inputs, n_steps=S,
                    trace=bool(int(os.environ.get("LSTM_TRACE", "0"))))
    return np.float32(total)



# revision 2
# speedup vs baseline: 1.0052x; 1.0052x over previous
"""AutoCompleteDecoderModel (LSTM enc-dec + CE loss) on 8 Trainium2 cores.

v2 strategy (B=256, S=512, H=512, V=128; 8 cores x 32 samples):
 - gates.T [2048, 32] per step in PSUM; gate m-tile order [g|i|f|o]; bank A
   holds m 0..7 (g,i), bank B m 8..15 (f,o).
 - Flights of 2 steps share one PSUM bank pair: the input projection
   (one-hot, bias folded) for both steps of a flight is ONE matmul per
   m-tile (N=64), cutting inproj matmul count 2x.
 - Recurrent matmuls use fp8e4 weights in DoubleRow mode: K=256 per
   instruction -> 32 matmuls/step instead of 64.  h state (hT=2h) is
   stored fp8e4; weights are scaled x64 (and i/f/o rows by an extra 0.5 so
   all four gates use a single tanh scale), unscaled in the ACT pre-scale.
 - Samples split into two anti-phase groups of 16: each group's activation
   chain (1 tanh [128,256], 3 STT, tanh(c), STT) overlaps the other group's
   matmuls, hiding the chain latency behind the recurrence of the peer.
 - Decoder tail batched over groups of 4 steps: logits.T via 4 matmuls
   (N=128) reading an hT ring, one exp [128,128], one tgt-dot mul, one
   ones-matmul reduce [1,256], one copy into the result accumulator.
 - Host: nll = ln(sumexp) - (tgtdot + proj_b[tgt]), masked mean, sum.
"""

import os
import sys

import numpy as np

if "/opt/trn_rl_repo" not in sys.path:
    sys.path.insert(0, "/opt/trn_rl_repo")

B, S, H, V = 256, 512, 512, 128
NCORES = 8
BS = B // NCORES   # 32 samples per core
U = 32             # steps per hw-loop iteration
FS = 2             # steps per PSUM flight
DG = 4             # steps per decoder tail group
WS = 64.0          # fp8 weight scale

_CACHE = {}

_PERM = None


def _perm():
    global _PERM
    if _PERM is None:
        _PERM = np.concatenate([
            np.arange(1024, 1536),  # g
            np.arange(0, 512),      # i
            np.arange(512, 1024),   # f
            np.arange(1536, 2048),  # o
        ])
    return _PERM


def _prep_weights(W_ih, W_hh, b_ih, b_hh):
    """Fold biases into W_ih, fold the hT=2h and single-tanh-scale factors,
    scale x64, quantize fp8e4, and pack for the kernel layouts."""
    import ml_dtypes

    fp8 = ml_dtypes.float8_e4m3
    perm = _perm()
    Wi = (np.asarray(W_ih, np.float64) + (np.asarray(b_ih, np.float64)
          + np.asarray(b_hh, np.float64))[:, None])[perm]  # [2048, 128]
    Wh = 0.5 * np.asarray(W_hh, np.float64)[perm]          # [2048, 512]
    Wi[512:] *= 0.5  # i,f,o rows: single tanh scale (tanh(z/2))
    Wh[512:] *= 0.5
    Wi *= WS
    Wh *= WS
    # input proj lhsT tiles: wih_t [V=128, 2048], m-tile m at cols m*128
    wih_t = np.ascontiguousarray(Wi.T).astype(fp8)
    # recurrent DoubleRow pairs: whh_dr [128, 8192], block (pk, m) at col
    # (pk*16+m)*256, within block [kk=2, c=128]; k-tile = 2*pk+kk
    Wt = np.ascontiguousarray(Wh.T)                        # [512, 2048]
    whh_dr = (Wt.reshape(2, 2, 128, 16, 128)               # [pk,kk,p,m,c]
              .transpose(2, 0, 3, 1, 4)                    # [p,pk,m,kk,c]
              .reshape(128, 8192).astype(fp8))
    return np.ascontiguousarray(wih_t), np.ascontiguousarray(whh_dr)


def _onehot_stream(idx):
    """idx [BS, S] int -> [128, S*32] fp8e4, col t*32+j = (idx[j,t]==v)."""
    import ml_dtypes
    oh = (np.arange(V, dtype=np.int32)[:, None, None]
          == np.asarray(idx, np.int32).T[None, :, :])  # [V, S, BS]
    return np.ascontiguousarray(
        oh.reshape(V, -1).astype(ml_dtypes.float8_e4m3))


def _build_module(n_steps, unrolled=False):
    import concourse.bacc as bacc
    import concourse.bass as bass
    import concourse.mybir as mybir
    import concourse.tile as tile

    f32 = mybir.dt.float32
    bf16 = mybir.dt.bfloat16
    fp8 = mybir.dt.float8e4
    AF = mybir.ActivationFunctionType
    OP = mybir.AluOpType
    PE = mybir.EngineType.PE
    DR = mybir.MatmulPerfMode.DoubleRow

    assert n_steps % U == 0
    n_iters = n_steps // U

    nc = bacc.Bacc("TRN2", target_bir_lowering=False, debug=False,
                   num_devices=NCORES)

    d_enc_whh = nc.dram_tensor("enc_whh", [128, 8192], fp8, kind="ExternalInput").ap()
    d_enc_wih = nc.dram_tensor("enc_wih", [128, 2048], fp8, kind="ExternalInput").ap()
    d_dec_whh = nc.dram_tensor("dec_whh", [128, 8192], fp8, kind="ExternalInput").ap()
    d_dec_wih = nc.dram_tensor("dec_wih", [128, 2048], fp8, kind="ExternalInput").ap()
    d_projt = nc.dram_tensor("projt", [128, 512], bf16, kind="ExternalInput").ap()
    d_projb = nc.dram_tensor("projb", [128, 1], f32, kind="ExternalInput").ap()
    d_enc_oh = nc.dram_tensor("enc_oh", [128, n_steps * BS], fp8, kind="ExternalInput").ap()
    d_dec_oh = nc.dram_tensor("dec_oh", [128, n_steps * BS], fp8, kind="ExternalInput").ap()
    d_res = nc.dram_tensor("res", [1, (n_steps // DG) * 256], f32, kind="ExternalOutput").ap()

    with tile.TileContext(nc) as tc:
        with (
            tc.tile_pool(name="const", bufs=1) as const_pool,
            tc.tile_pool(name="oh", bufs=2) as oh_pool,
            tc.tile_pool(name="pgA", bufs=3, space="PSUM") as pgA_pool,
            tc.tile_pool(name="pgB", bufs=3, space="PSUM") as pgB_pool,
            tc.tile_pool(name="plog", bufs=1, space="PSUM") as plog_pool,
            tc.tile_pool(name="prsp", bufs=1, space="PSUM") as prsp_pool,
            tc.tile_pool(name="work", bufs=3) as work_pool,
            tc.tile_pool(name="stack", bufs=2) as stack_pool,
            tc.tile_pool(name="acc", bufs=2) as acc_pool,
        ):
            w_enc_hh = const_pool.tile([128, 8192], fp8, tag="wehh")
            w_enc_ih = const_pool.tile([128, 2048], fp8, tag="weih")
            w_dec_hh = const_pool.tile([128, 8192], fp8, tag="wdhh")
            w_dec_ih = const_pool.tile([128, 2048], fp8, tag="wdih")
            w_projt = const_pool.tile([128, 512], bf16, tag="wpt")
            w_projb = const_pool.tile([128, 1], f32, tag="wpb")
            ones_col = const_pool.tile([128, 1], bf16, tag="ones")
            sstA = const_pool.tile([128, 64], f32, tag="sstA")
            sstB = const_pool.tile([128, 64], f32, tag="sstB")

            nc.sync.dma_start(w_enc_hh[:], d_enc_whh)
            nc.sync.dma_start(w_enc_ih[:], d_enc_wih)
            nc.sync.dma_start(w_dec_hh[:], d_dec_whh)
            nc.sync.dma_start(w_dec_ih[:], d_dec_wih)
            nc.sync.dma_start(w_projt[:], d_projt)
            nc.sync.dma_start(w_projb[:], d_projb)
            nc.vector.memset(ones_col[:], 1.0)
            nc.vector.memset(sstA[:], 0.0)
            nc.vector.memset(sstB[:], 0.0)
            # hT rings (one per sample group): slot v holds hT of step u with
            # u%DG == v, col k*16+j.  First step reads slot DG-1 (h0 = 0).
            ringA = const_pool.tile([128, DG * 64], fp8, tag="ringA")
            ringB = const_pool.tile([128, DG * 64], fp8, tag="ringB")
            nc.vector.memset(ringA[:], 0.0)
            nc.vector.memset(ringB[:], 0.0)
            rings = (ringA, ringB)
            ssts = (sstA, sstB)

            def inproj(w_ih, psq, ohq, t, q):
                """Gate-PSUM init, one group: psq col = m*16 + j."""
                for m in range(16):
                    nc.tensor.matmul(
                        psq[:, m * 16:(m + 1) * 16],
                        w_ih[:, m * 128:(m + 1) * 128],
                        ohq[:, t * BS + q * 16: t * BS + q * 16 + 16],
                        start=True, stop=False, skip_group_check=True)

            def rec_mms(w_hh, psq, hprev):
                """DoubleRow K=256 recurrent matmuls for one sample group."""
                for m in range(16):
                    out = psq[:, m * 16:(m + 1) * 16]
                    for pk in range(2):
                        w = w_hh[:, (pk * 16 + m) * 256:(pk * 16 + m + 1) * 256]
                        nc.tensor.matmul(
                            out,
                            w.rearrange("p (k c) -> p k c", k=2),
                            hprev[:, pk * 32:(pk + 1) * 32]
                            .rearrange("p (k j) -> p k j", k=2),
                            start=False, stop=(pk == 1),
                            perf_mode=DR, skip_group_check=True)

            def chain(psqs, v):
                """Both groups' activation chains, stage-interleaved."""
                Ts, tc2s = [], []
                for q in range(2):
                    T = work_pool.tile([128, 256], bf16, tag=f"T{q}",
                                       name=f"T{q}")
                    nc.scalar.activation(T[:], psqs[q][:, :], AF.Tanh,
                                         scale=1.0 / WS)
                    Ts.append(T)
                for q in range(2):
                    T, sst_q = Ts[q], ssts[q]
                    a2 = work_pool.tile([128, 64], f32, tag=f"a2{q}",
                                        name=f"a2{q}")
                    nc.vector.scalar_tensor_tensor(a2[:], T[:, 128:192], 1.0,
                                                   sst_q[:], OP.add, OP.mult)
                    a1 = work_pool.tile([128, 64], f32, tag=f"a1{q}",
                                        name=f"a1{q}")
                    nc.vector.scalar_tensor_tensor(a1[:], T[:, 64:128], 1.0,
                                                   T[:, 0:64], OP.add, OP.mult)
                    nc.vector.scalar_tensor_tensor(sst_q[:], a2[:], 0.5,
                                                   a1[:], OP.mult, OP.add)
                for q in range(2):
                    tc2 = work_pool.tile([128, 64], bf16, tag=f"tc2{q}",
                                         name=f"tc2{q}")
                    nc.scalar.activation(tc2[:], ssts[q][:], AF.Tanh, scale=0.5)
                    tc2s.append(tc2)
                for q in range(2):
                    nc.vector.scalar_tensor_tensor(
                        rings[q][:, v * 64:(v + 1) * 64], Ts[q][:, 192:256],
                        1.0, tc2s[q][:], OP.add, OP.mult)

            def dec_tail_mm(g):
                """Batched logits matmuls for steps DG*g .. DG*g+3."""
                ps_l = plog_pool.tile([128, 128], f32, tag="psl")
                for q in range(2):
                    hsrc = rings[q][:, :].rearrange("p (u k j) -> p u k j",
                                                    u=DG, k=4, j=16)
                    for k in range(4):
                        nc.tensor.matmul(ps_l[:, q * 64:(q + 1) * 64],
                                         w_projt[:, k * 128:(k + 1) * 128],
                                         hsrc[:, :, k, :],
                                         start=(k == 0), stop=(k == 3),
                                         skip_group_check=True)
                return ps_l

            def dec_tail_reduce(ps_l, ohq, gl, accum, g):
                """exp / tgt-dot stack (bf16), ones-matmul, accum copy."""
                st = stack_pool.tile([128, 256], bf16, tag="st")
                nc.scalar.activation(st[:, 0:128], ps_l[:, :], AF.Exp,
                                     bias=w_projb[:, 0:1], scale=1.0)
                ohv = ohq[:, :].rearrange("p (t r) -> p t r", r=32)
                for q in range(2):
                    nc.vector.tensor_mul(
                        st[:, 128 + q * 64:128 + (q + 1) * 64]
                        .rearrange("p (v j) -> p v j", v=DG),
                        ps_l[:, q * 64:(q + 1) * 64]
                        .rearrange("p (v j) -> p v j", v=DG),
                        ohv[:, gl * DG:(gl + 1) * DG, q * 16:(q + 1) * 16])
                rp = prsp_pool.tile([1, 256], f32, tag="rp")
                nc.tensor.matmul(rp[:, :], ones_col[:, 0:1], st[:, :],
                                 start=True, stop=True)
                nc.vector.tensor_copy(accum[:, g * 256:(g + 1) * 256],
                                      rp[:, :])

            def body(i, w_hh, w_ih, d_oh, dec):
                ohq = oh_pool.tile([128, U * BS], fp8, tag="oh")
                nc.sync.dma_start(ohq[:], d_oh[:, bass.ts(i, U * BS)])
                accum = None
                if dec:
                    accum = acc_pool.tile([1, (U // DG) * 256], f32, tag="accum")
                psA = pgA_pool.tile([128, 256], f32, tag="psA")
                psB = pgB_pool.tile([128, 256], f32, tag="psB")
                inproj(w_ih, psA, ohq, 0, 0)
                inproj(w_ih, psB, ohq, 0, 1)
                pend = None
                for u in range(U):
                    g, v = u // DG, u % DG
                    pv = (u - 1) % DG
                    psqs = (psA, psB)
                    rec_mms(w_hh, psA, rings[0][:, pv * 64:(pv + 1) * 64])
                    rec_mms(w_hh, psB, rings[1][:, pv * 64:(pv + 1) * 64])
                    if u + 1 < U:
                        psA_n = pgA_pool.tile([128, 256], f32, tag="psA")
                        psB_n = pgB_pool.tile([128, 256], f32, tag="psB")
                        inproj(w_ih, psA_n, ohq, u + 1, 0)
                        inproj(w_ih, psB_n, ohq, u + 1, 1)
                    chain(psqs, v)
                    if pend is not None:
                        dec_tail_reduce(*pend)
                        pend = None
                    if dec and v == DG - 1:
                        pend = (dec_tail_mm(g), ohq, g, accum, g)
                    if u + 1 < U:
                        psA, psB = psA_n, psB_n
                if pend is not None:
                    dec_tail_reduce(*pend)
                if dec:
                    nc.sync.dma_start(d_res[:, bass.ts(i, (U // DG) * 256)],
                                      accum[:])

            if unrolled:
                for i in range(n_iters):
                    body(i, w_enc_hh, w_enc_ih, d_enc_oh, False)
                for i in range(n_iters):
                    body(i, w_dec_hh, w_dec_ih, d_dec_oh, True)
            else:
                with tc.For_i(0, n_iters, 1, hint_engines=(PE,), name="enc") as i:
                    body(i, w_enc_hh, w_enc_ih, d_enc_oh, False)
                with tc.For_i(0, n_iters, 1, hint_engines=(PE,), name="dec") as i:
                    body(i, w_dec_hh, w_dec_ih, d_dec_oh, True)

    nc.compile()
    return nc


def _run(inputs, n_steps=S, trace=False):
    from concourse import bass_utils

    key = n_steps
    if key not in _CACHE:
        _CACHE[key] = _build_module(n_steps)
    nc = _CACHE[key]

    enc_wih, enc_whh = _prep_weights(inputs["enc_W_ih"], inputs["enc_W_hh"],
                                     inputs["enc_b_ih"], inputs["enc_b_hh"])
    dec_wih, dec_whh = _prep_weights(inputs["dec_W_ih"], inputs["dec_W_hh"],
                                     inputs["dec_b_ih"], inputs["dec_b_hh"])
    import ml_dtypes
    projW = 0.5 * np.asarray(inputs["proj_W"], np.float64)  # [128, 512]
    projt = (np.ascontiguousarray(projW.T).reshape(4, 128, 128)
             .transpose(1, 0, 2).reshape(128, 512).astype(ml_dtypes.bfloat16))
    projb = np.ascontiguousarray(
        np.asarray(inputs["proj_b"], np.float32).reshape(128, 1))

    C_idx = np.asarray(inputs["C_idx"])[:, :n_steps]
    E = np.asarray(inputs["E"])
    Etgt = E[:, :n_steps]

    in_maps = []
    for c in range(NCORES):
        sl = slice(c * BS, (c + 1) * BS)
        in_maps.append({
            "enc_whh": enc_whh, "enc_wih": enc_wih,
            "dec_whh": dec_whh, "dec_wih": dec_wih,
            "projt": np.ascontiguousarray(projt), "projb": projb,
            "enc_oh": _onehot_stream(C_idx[sl]),
            "dec_oh": _onehot_stream(Etgt[sl]),
        })

    res = bass_utils.run_bass_kernel_spmd(
        nc, in_maps, core_ids=list(range(NCORES)), trace=trace,
        trace_cores=[0] if trace else None)

    # ---- host-side loss assembly (float64) ----
    proj_b = np.asarray(inputs["proj_b"], np.float64)
    nll = np.empty((B, n_steps), np.float64)
    for c in range(NCORES):
        r = np.asarray(res.results[c]["res"], np.float64).reshape(
            n_steps // DG, 2, 2, DG, 16)       # [g, {sumexp,tgtdot}, q, v, j]
        r = r.transpose(0, 1, 3, 2, 4)         # [g, s, v, q, j]
        sumexp = r[:, 0].reshape(n_steps, BS)  # [u, sample 16q+j]
        tgtdot = r[:, 1].reshape(n_steps, BS)
        tgt = Etgt[c * BS:(c + 1) * BS]            # [j, u]
        bias_t = proj_b[tgt]                       # [j, u]
        nll[c * BS:(c + 1) * BS] = (np.log(sumexp).T
                                    - (tgtdot.T + bias_t))
    mask = (Etgt != 0).astype(np.float64)          # [B, u]
    num = (nll * mask).sum(axis=0)
    cnt = mask.sum(axis=0)
    step_loss = np.where(cnt > 0, num / np.maximum(cnt, 1.0), 0.0)
    total = np.float32(step_loss.sum())
    return total, res


def kernel(**inputs) -> np.ndarray:
    total, _ = _run(inputs, n_steps=S,
                    trace=bool(int(os.environ.get("LSTM_TRACE", "0"))))
    return np.float32(total)


# revision 4
# speedup vs baseline: 1.0670x; 1.0615x over previous
"""AutoCompleteDecoderModel (LSTM enc-dec + CE loss) on 8 Trainium2 cores.

v2 strategy (B=256, S=512, H=512, V=128; 8 cores x 32 samples):
 - gates.T [2048, 32] per step in PSUM; gate m-tile order [g|i|f|o]; bank A
   holds m 0..7 (g,i), bank B m 8..15 (f,o).
 - Flights of 2 steps share one PSUM bank pair: the input projection
   (one-hot, bias folded) for both steps of a flight is ONE matmul per
   m-tile (N=64), cutting inproj matmul count 2x.
 - Recurrent matmuls use fp8e4 weights in DoubleRow mode: K=256 per
   instruction -> 32 matmuls/step instead of 64.  h state (hT=2h) is
   stored fp8e4; weights are scaled x64 (and i/f/o rows by an extra 0.5 so
   all four gates use a single tanh scale), unscaled in the ACT pre-scale.
 - Samples split into two anti-phase groups of 16: each group's activation
   chain (1 tanh [128,256], 3 STT, tanh(c), STT) overlaps the other group's
   matmuls, hiding the chain latency behind the recurrence of the peer.
 - Decoder tail batched over groups of 4 steps: logits.T via 4 matmuls
   (N=128) reading an hT ring, one exp [128,128], one tgt-dot mul, one
   ones-matmul reduce [1,256], one copy into the result accumulator.
 - Host: nll = ln(sumexp) - (tgtdot + proj_b[tgt]), masked mean, sum.
"""

import os
import sys

import numpy as np

if "/opt/trn_rl_repo" not in sys.path:
    sys.path.insert(0, "/opt/trn_rl_repo")

B, S, H, V = 256, 512, 512, 128
NCORES = 8
BS = B // NCORES   # 32 samples per core
U = 64             # steps per hw-loop iteration
FS = 2             # steps per PSUM flight
DG = 4             # steps per decoder tail group
WS = 64.0          # fp8 weight scale

_CACHE = {}

_PERM = None


def _perm():
    global _PERM
    if _PERM is None:
        _PERM = np.concatenate([
            np.arange(1024, 1536),  # g
            np.arange(0, 512),      # i
            np.arange(512, 1024),   # f
            np.arange(1536, 2048),  # o
        ])
    return _PERM


def _prep_weights(W_ih, W_hh, b_ih, b_hh):
    """Fold biases into W_ih, fold the hT=2h and single-tanh-scale factors,
    scale x64, quantize fp8e4, and pack for the kernel layouts."""
    import ml_dtypes

    fp8 = ml_dtypes.float8_e4m3
    perm = _perm()
    Wi = (np.asarray(W_ih, np.float64) + (np.asarray(b_ih, np.float64)
          + np.asarray(b_hh, np.float64))[:, None])[perm]  # [2048, 128]
    Wh = 0.5 * np.asarray(W_hh, np.float64)[perm]          # [2048, 512]
    Wi[512:] *= 0.5  # i,f,o rows: single tanh scale (tanh(z/2))
    Wh[512:] *= 0.5
    Wi *= WS
    Wh *= WS
    # input proj lhsT tiles: wih_t [V=128, 2048], m-tile m at cols m*128
    wih_t = np.ascontiguousarray(Wi.T).astype(fp8)
    # recurrent DoubleRow pairs: whh_dr [128, 8192], block (pk, m) at col
    # (pk*16+m)*256, within block [kk=2, c=128]; k-tile = 2*pk+kk
    Wt = np.ascontiguousarray(Wh.T)                        # [512, 2048]
    whh_dr = (Wt.reshape(2, 2, 128, 16, 128)               # [pk,kk,p,m,c]
              .transpose(2, 0, 3, 1, 4)                    # [p,pk,m,kk,c]
              .reshape(128, 8192).astype(fp8))
    return np.ascontiguousarray(wih_t), np.ascontiguousarray(whh_dr)


def _onehot_stream(idx):
    """idx [BS, S] int -> [128, S*32] fp8e4, col t*32+j = (idx[j,t]==v)."""
    import ml_dtypes
    oh = (np.arange(V, dtype=np.int32)[:, None, None]
          == np.asarray(idx, np.int32).T[None, :, :])  # [V, S, BS]
    return np.ascontiguousarray(
        oh.reshape(V, -1).astype(ml_dtypes.float8_e4m3))


def _build_module(n_steps, unrolled=False):
    import concourse.bacc as bacc
    import concourse.bass as bass
    import concourse.mybir as mybir
    import concourse.tile as tile

    f32 = mybir.dt.float32
    bf16 = mybir.dt.bfloat16
    fp8 = mybir.dt.float8e4
    AF = mybir.ActivationFunctionType
    OP = mybir.AluOpType
    PE = mybir.EngineType.PE
    DR = mybir.MatmulPerfMode.DoubleRow

    assert n_steps % U == 0
    n_iters = n_steps // U

    nc = bacc.Bacc("TRN2", target_bir_lowering=False, debug=False,
                   num_devices=NCORES)

    d_enc_whh = nc.dram_tensor("enc_whh", [128, 8192], fp8, kind="ExternalInput").ap()
    d_enc_wih = nc.dram_tensor("enc_wih", [128, 2048], fp8, kind="ExternalInput").ap()
    d_dec_whh = nc.dram_tensor("dec_whh", [128, 8192], fp8, kind="ExternalInput").ap()
    d_dec_wih = nc.dram_tensor("dec_wih", [128, 2048], fp8, kind="ExternalInput").ap()
    d_projt = nc.dram_tensor("projt", [128, 512], bf16, kind="ExternalInput").ap()
    d_projb = nc.dram_tensor("projb", [128, 1], f32, kind="ExternalInput").ap()
    d_enc_oh = nc.dram_tensor("enc_oh", [128, n_steps * BS], fp8, kind="ExternalInput").ap()
    d_dec_oh = nc.dram_tensor("dec_oh", [128, n_steps * BS], fp8, kind="ExternalInput").ap()
    d_res = nc.dram_tensor("res", [1, (n_steps // DG) * 256], f32, kind="ExternalOutput").ap()

    with tile.TileContext(nc) as tc:
        with (
            tc.tile_pool(name="const", bufs=1) as const_pool,
            tc.tile_pool(name="oh", bufs=3) as oh_pool,
            tc.tile_pool(name="pgA", bufs=3, space="PSUM") as pgA_pool,
            tc.tile_pool(name="pgB", bufs=3, space="PSUM") as pgB_pool,
            tc.tile_pool(name="plog", bufs=1, space="PSUM") as plog_pool,
            tc.tile_pool(name="prsp", bufs=1, space="PSUM") as prsp_pool,
            tc.tile_pool(name="work", bufs=3) as work_pool,
            tc.tile_pool(name="stack", bufs=2) as stack_pool,
            tc.tile_pool(name="acc", bufs=2) as acc_pool,
        ):
            w_enc_hh = const_pool.tile([128, 8192], fp8, tag="wehh")
            w_enc_ih = const_pool.tile([128, 2048], fp8, tag="weih")
            w_dec_hh = const_pool.tile([128, 8192], fp8, tag="wdhh")
            w_dec_ih = const_pool.tile([128, 2048], fp8, tag="wdih")
            w_projt = const_pool.tile([128, 512], bf16, tag="wpt")
            w_projb = const_pool.tile([128, 1], f32, tag="wpb")
            ones_col = const_pool.tile([128, 1], bf16, tag="ones")
            sstA = const_pool.tile([128, 64], f32, tag="sstA")
            sstB = const_pool.tile([128, 64], f32, tag="sstB")

            nc.sync.dma_start(w_enc_hh[:], d_enc_whh)
            nc.sync.dma_start(w_enc_ih[:], d_enc_wih)
            nc.sync.dma_start(w_dec_hh[:], d_dec_whh)
            nc.sync.dma_start(w_dec_ih[:], d_dec_wih)
            nc.sync.dma_start(w_projt[:], d_projt)
            nc.sync.dma_start(w_projb[:], d_projb)
            nc.vector.memset(ones_col[:], 1.0)
            nc.vector.memset(sstA[:], 0.0)
            nc.vector.memset(sstB[:], 0.0)
            # hT rings: per sample group, per k-half (k01 / k23) so the
            # pk0 recurrent matmuls only wait on the first half of hT.
            # Slot v holds hT of step u with u%DG == v, col k*16+j (2 k each).
            ringA0 = const_pool.tile([128, DG * 32], fp8, tag="ringA0")
            ringA1 = const_pool.tile([128, DG * 32], fp8, tag="ringA1")
            ringB0 = const_pool.tile([128, DG * 32], fp8, tag="ringB0")
            ringB1 = const_pool.tile([128, DG * 32], fp8, tag="ringB1")
            for r in (ringA0, ringA1, ringB0, ringB1):
                nc.vector.memset(r[:], 0.0)
            rings = ((ringA0, ringA1), (ringB0, ringB1))
            ssts = (sstA, sstB)

            def inproj(w_ih, psq, ohq, t, q):
                """Gate-PSUM init, one group: psq col = m*16 + j."""
                for m in range(16):
                    nc.tensor.matmul(
                        psq[:, m * 16:(m + 1) * 16],
                        w_ih[:, m * 128:(m + 1) * 128],
                        ohq[:, t * BS + q * 16: t * BS + q * 16 + 16],
                        start=True, stop=False, skip_group_check=True)

            def rec_mms(w_hh, psq, q, pv):
                """DoubleRow K=256 recurrent matmuls for one sample group.
                pk0 first across all m (it only needs the k01 ring half)."""
                for pk in range(2):
                    hprev = rings[q][pk][:, pv * 32:(pv + 1) * 32]
                    for m in range(16):
                        out = psq[:, m * 16:(m + 1) * 16]
                        w = w_hh[:, (pk * 16 + m) * 256:(pk * 16 + m + 1) * 256]
                        nc.tensor.matmul(
                            out,
                            w.rearrange("p (k c) -> p k c", k=2),
                            hprev.rearrange("p (k j) -> p k j", k=2),
                            start=False, stop=(pk == 1),
                            perf_mode=DR, skip_group_check=True)

            def chain(psqs, v):
                """Both groups' activation chains, stage-interleaved."""
                Ts, tc2s = [], []
                for q in range(2):
                    T = work_pool.tile([128, 256], bf16, tag=f"T{q}",
                                       name=f"T{q}")
                    nc.scalar.activation(T[:], psqs[q][:, :], AF.Tanh,
                                         scale=1.0 / WS)
                    Ts.append(T)
                for q in range(2):
                    T, sst_q = Ts[q], ssts[q]
                    a2 = work_pool.tile([128, 64], f32, tag=f"a2{q}",
                                        name=f"a2{q}")
                    nc.vector.scalar_tensor_tensor(a2[:], T[:, 128:192], 1.0,
                                                   sst_q[:], OP.add, OP.mult)
                    a1 = work_pool.tile([128, 64], f32, tag=f"a1{q}",
                                        name=f"a1{q}")
                    nc.vector.scalar_tensor_tensor(a1[:], T[:, 64:128], 1.0,
                                                   T[:, 0:64], OP.add, OP.mult)
                    nc.vector.scalar_tensor_tensor(sst_q[:], a2[:], 0.5,
                                                   a1[:], OP.mult, OP.add)
                for q in range(2):
                    tc2 = work_pool.tile([128, 64], bf16, tag=f"tc2{q}",
                                         name=f"tc2{q}")
                    nc.scalar.activation(tc2[:], ssts[q][:], AF.Tanh, scale=0.5)
                    tc2s.append(tc2)
                for q in range(2):
                    for h in range(2):
                        nc.vector.scalar_tensor_tensor(
                            rings[q][h][:, v * 32:(v + 1) * 32],
                            Ts[q][:, 192 + h * 32:192 + (h + 1) * 32],
                            1.0, tc2s[q][:, h * 32:(h + 1) * 32],
                            OP.add, OP.mult)

            def dec_tail_mm(g):
                """Batched logits matmuls for steps DG*g .. DG*g+3."""
                ps_l = plog_pool.tile([128, 128], f32, tag="psl")
                for q in range(2):
                    for k in range(4):
                        hsrc = rings[q][k // 2][:, :].rearrange(
                            "p (u k j) -> p u k j", u=DG, k=2, j=16)
                        nc.tensor.matmul(ps_l[:, q * 64:(q + 1) * 64],
                                         w_projt[:, k * 128:(k + 1) * 128],
                                         hsrc[:, :, k % 2, :],
                                         start=(k == 0), stop=(k == 3),
                                         skip_group_check=True)
                return ps_l

            def dec_tail_reduce(ps_l, ohq, gl, accum, g, i_sym):
                """exp / tgt-dot stack (bf16), ones-matmul, accum copy."""
                st = stack_pool.tile([128, 256], bf16, tag="st")
                nc.scalar.activation(st[:, 0:128], ps_l[:, :], AF.Exp,
                                     bias=w_projb[:, 0:1], scale=1.0)
                ohv = ohq[:, :].rearrange("p (t r) -> p t r", r=32)
                for q in range(2):
                    nc.vector.tensor_mul(
                        st[:, 128 + q * 64:128 + (q + 1) * 64]
                        .rearrange("p (v j) -> p v j", v=DG),
                        ps_l[:, q * 64:(q + 1) * 64]
                        .rearrange("p (v j) -> p v j", v=DG),
                        ohv[:, gl * DG:(gl + 1) * DG, q * 16:(q + 1) * 16])
                rp = prsp_pool.tile([1, 256], f32, tag="rp")
                nc.tensor.matmul(rp[:, :], ones_col[:, 0:1], st[:, :],
                                 start=True, stop=True)
                nc.vector.tensor_copy(accum[:, g * 256:(g + 1) * 256],
                                      rp[:, :])

            def body(i, w_hh, w_ih, d_oh, dec):
                ohq = oh_pool.tile([128, U * BS], fp8, tag="oh")
                nc.sync.dma_start(ohq[:], d_oh[:, bass.ts(i, U * BS)])
                accum = None
                if dec:
                    accum = acc_pool.tile([1, (U // DG) * 256], f32, tag="accum")
                psA = pgA_pool.tile([128, 256], f32, tag="psA")
                psB = pgB_pool.tile([128, 256], f32, tag="psB")
                inproj(w_ih, psA, ohq, 0, 0)
                inproj(w_ih, psB, ohq, 0, 1)
                pend_mm = None
                pend_red = None
                for u in range(U):
                    g, v = u // DG, u % DG
                    pv = (u - 1) % DG
                    psqs = (psA, psB)
                    rec_mms(w_hh, psA, 0, pv)
                    rec_mms(w_hh, psB, 1, pv)
                    # proj matmuls for the previous dec group run here: after
                    # this step's recurrent matmuls (so they don't delay them
                    # in the PE queue) but before this step's hT overwrites
                    # ring slot 0.
                    if pend_mm is not None:
                        pend_red = (dec_tail_mm(pend_mm), ohq, pend_mm,
                                    accum, pend_mm, i)
                        pend_mm = None
                    if u + 1 < U:
                        psA_n = pgA_pool.tile([128, 256], f32, tag="psA")
                        psB_n = pgB_pool.tile([128, 256], f32, tag="psB")
                        inproj(w_ih, psA_n, ohq, u + 1, 0)
                        inproj(w_ih, psB_n, ohq, u + 1, 1)
                    chain(psqs, v)
                    if pend_red is not None:
                        dec_tail_reduce(*pend_red)
                        pend_red = None
                    if dec and v == DG - 1:
                        pend_mm = g
                    if u + 1 < U:
                        psA, psB = psA_n, psB_n
                if pend_mm is not None:
                    dec_tail_reduce(dec_tail_mm(pend_mm), ohq, pend_mm,
                                    accum, pend_mm, i)
                if dec:
                    nc.sync.dma_start(d_res[:, bass.ts(i, (U // DG) * 256)],
                                      accum[:])

            if unrolled:
                for i in range(n_iters):
                    body(i, w_enc_hh, w_enc_ih, d_enc_oh, False)
                for i in range(n_iters):
                    body(i, w_dec_hh, w_dec_ih, d_dec_oh, True)
            else:
                with tc.For_i(0, n_iters, 1, hint_engines=(PE,), name="enc") as i:
                    body(i, w_enc_hh, w_enc_ih, d_enc_oh, False)
                with tc.For_i(0, n_iters, 1, hint_engines=(PE,), name="dec") as i:
                    body(i, w_dec_hh, w_dec_ih, d_dec_oh, True)

    nc.compile()
    return nc


def _run(inputs, n_steps=S, trace=False):
    from concourse import bass_utils

    key = n_steps
    if key not in _CACHE:
        _CACHE[key] = _build_module(n_steps)
    nc = _CACHE[key]

    enc_wih, enc_whh = _prep_weights(inputs["enc_W_ih"], inputs["enc_W_hh"],
                                     inputs["enc_b_ih"], inputs["enc_b_hh"])
    dec_wih, dec_whh = _prep_weights(inputs["dec_W_ih"], inputs["dec_W_hh"],
                                     inputs["dec_b_ih"], inputs["dec_b_hh"])
    import ml_dtypes
    projW = 0.5 * np.asarray(inputs["proj_W"], np.float64)  # [128, 512]
    projt = (np.ascontiguousarray(projW.T).reshape(4, 128, 128)
             .transpose(1, 0, 2).reshape(128, 512).astype(ml_dtypes.bfloat16))
    projb = np.ascontiguousarray(
        np.asarray(inputs["proj_b"], np.float32).reshape(128, 1))

    C_idx = np.asarray(inputs["C_idx"])[:, :n_steps]
    E = np.asarray(inputs["E"])
    Etgt = E[:, :n_steps]

    in_maps = []
    for c in range(NCORES):
        sl = slice(c * BS, (c + 1) * BS)
        in_maps.append({
            "enc_whh": enc_whh, "enc_wih": enc_wih,
            "dec_whh": dec_whh, "dec_wih": dec_wih,
            "projt": np.ascontiguousarray(projt), "projb": projb,
            "enc_oh": _onehot_stream(C_idx[sl]),
            "dec_oh": _onehot_stream(Etgt[sl]),
        })

    res = bass_utils.run_bass_kernel_spmd(
        nc, in_maps, core_ids=list(range(NCORES)), trace=trace,
        trace_cores=[0] if trace else None)

    # ---- host-side loss assembly (float64) ----
    proj_b = np.asarray(inputs["proj_b"], np.float64)
    nll = np.empty((B, n_steps), np.float64)
    for c in range(NCORES):
        r = np.asarray(res.results[c]["res"], np.float64).reshape(
            n_steps // DG, 2, 2, DG, 16)       # [g, {sumexp,tgtdot}, q, v, j]
        r = r.transpose(0, 1, 3, 2, 4)         # [g, s, v, q, j]
        sumexp = r[:, 0].reshape(n_steps, BS)  # [u, sample 16q+j]
        tgtdot = r[:, 1].reshape(n_steps, BS)
        tgt = Etgt[c * BS:(c + 1) * BS]            # [j, u]
        bias_t = proj_b[tgt]                       # [j, u]
        nll[c * BS:(c + 1) * BS] = (np.log(sumexp).T
                                    - (tgtdot.T + bias_t))
    mask = (Etgt != 0).astype(np.float64)          # [B, u]
    num = (nll * mask).sum(axis=0)
    cnt = mask.sum(axis=0)
    step_loss = np.where(cnt > 0, num / np.maximum(cnt, 1.0), 0.0)
    total = np.float32(step_loss.sum())
    return total, res


def kernel(**inputs) -> np.ndarray:
    total, _ = _run(inputs, n_steps=S,
                    trace=bool(int(os.environ.get("LSTM_TRACE", "0"))))
    return np.float32(total)


# revision 5
# speedup vs baseline: 1.0808x; 1.0129x over previous
"""AutoCompleteDecoderModel (LSTM enc-dec + CE loss) on 8 Trainium2 cores.

v2 strategy (B=256, S=512, H=512, V=128; 8 cores x 32 samples):
 - gates.T [2048, 32] per step in PSUM; gate m-tile order [g|i|f|o]; bank A
   holds m 0..7 (g,i), bank B m 8..15 (f,o).
 - Flights of 2 steps share one PSUM bank pair: the input projection
   (one-hot, bias folded) for both steps of a flight is ONE matmul per
   m-tile (N=64), cutting inproj matmul count 2x.
 - Recurrent matmuls use fp8e4 weights in DoubleRow mode: K=256 per
   instruction -> 32 matmuls/step instead of 64.  h state (hT=2h) is
   stored fp8e4; weights are scaled x64 (and i/f/o rows by an extra 0.5 so
   all four gates use a single tanh scale), unscaled in the ACT pre-scale.
 - Samples split into two anti-phase groups of 16: each group's activation
   chain (1 tanh [128,256], 3 STT, tanh(c), STT) overlaps the other group's
   matmuls, hiding the chain latency behind the recurrence of the peer.
 - Decoder tail batched over groups of 4 steps: logits.T via 4 matmuls
   (N=128) reading an hT ring, one exp [128,128], one tgt-dot mul, one
   ones-matmul reduce [1,256], one copy into the result accumulator.
 - Host: nll = ln(sumexp) - (tgtdot + proj_b[tgt]), masked mean, sum.
"""

import os
import sys

import numpy as np

if "/opt/trn_rl_repo" not in sys.path:
    sys.path.insert(0, "/opt/trn_rl_repo")

B, S, H, V = 256, 512, 512, 128
NCORES = 8
BS = B // NCORES   # 32 samples per core
U = 64             # steps per hw-loop iteration
FS = 2             # steps per PSUM flight
DG = 4             # steps per decoder tail group
WS = 64.0          # fp8 weight scale

_CACHE = {}

_PERM = None


def _perm():
    global _PERM
    if _PERM is None:
        _PERM = np.concatenate([
            np.arange(1024, 1536),  # g
            np.arange(0, 512),      # i
            np.arange(512, 1024),   # f
            np.arange(1536, 2048),  # o
        ])
    return _PERM


def _prep_weights(W_ih, W_hh, b_ih, b_hh):
    """Fold biases into W_ih, fold the hT=2h and single-tanh-scale factors,
    scale x64, quantize fp8e4, and pack for the kernel layouts."""
    import ml_dtypes

    fp8 = ml_dtypes.float8_e4m3
    perm = _perm()
    Wi = (np.asarray(W_ih, np.float64) + (np.asarray(b_ih, np.float64)
          + np.asarray(b_hh, np.float64))[:, None])[perm]  # [2048, 128]
    Wh = 0.5 * np.asarray(W_hh, np.float64)[perm]          # [2048, 512]
    Wi[512:] *= 0.5  # i,f,o rows: single tanh scale (tanh(z/2))
    Wh[512:] *= 0.5
    Wi *= WS
    Wh *= WS
    # input proj lhsT tiles: wih_t [V=128, 2048], m-tile m at cols m*128
    wih_t = np.ascontiguousarray(Wi.T).astype(fp8)
    # recurrent DoubleRow pairs: whh_dr [128, 8192], block (pk, m) at col
    # (pk*16+m)*256, within block [kk=2, c=128]; k-tile = 2*pk+kk
    Wt = np.ascontiguousarray(Wh.T)                        # [512, 2048]
    whh_dr = (Wt.reshape(2, 2, 128, 16, 128)               # [pk,kk,p,m,c]
              .transpose(2, 0, 3, 1, 4)                    # [p,pk,m,kk,c]
              .reshape(128, 8192).astype(fp8))
    return np.ascontiguousarray(wih_t), np.ascontiguousarray(whh_dr)


def _onehot_stream(idx):
    """idx [BS, S] int -> [128, S*32] fp8e4, col t*32+j = (idx[j,t]==v)."""
    import ml_dtypes
    oh = (np.arange(V, dtype=np.int32)[:, None, None]
          == np.asarray(idx, np.int32).T[None, :, :])  # [V, S, BS]
    return np.ascontiguousarray(
        oh.reshape(V, -1).astype(ml_dtypes.float8_e4m3))


def _build_module(n_steps, unrolled=False):
    import concourse.bacc as bacc
    import concourse.bass as bass
    import concourse.mybir as mybir
    import concourse.tile as tile

    f32 = mybir.dt.float32
    bf16 = mybir.dt.bfloat16
    fp8 = mybir.dt.float8e4
    AF = mybir.ActivationFunctionType
    OP = mybir.AluOpType
    PE = mybir.EngineType.PE
    DR = mybir.MatmulPerfMode.DoubleRow

    assert n_steps % U == 0
    n_iters = n_steps // U

    nc = bacc.Bacc("TRN2", target_bir_lowering=False, debug=False,
                   num_devices=NCORES)

    d_enc_whh = nc.dram_tensor("enc_whh", [128, 8192], fp8, kind="ExternalInput").ap()
    d_enc_wih = nc.dram_tensor("enc_wih", [128, 2048], fp8, kind="ExternalInput").ap()
    d_dec_whh = nc.dram_tensor("dec_whh", [128, 8192], fp8, kind="ExternalInput").ap()
    d_dec_wih = nc.dram_tensor("dec_wih", [128, 2048], fp8, kind="ExternalInput").ap()
    d_projt = nc.dram_tensor("projt", [128, 512], bf16, kind="ExternalInput").ap()
    d_projb = nc.dram_tensor("projb", [128, 1], f32, kind="ExternalInput").ap()
    d_enc_oh = nc.dram_tensor("enc_oh", [128, n_steps * BS], fp8, kind="ExternalInput").ap()
    d_dec_oh = nc.dram_tensor("dec_oh", [128, n_steps * BS], fp8, kind="ExternalInput").ap()
    d_res = nc.dram_tensor("res", [1, (n_steps // DG) * 256], f32, kind="ExternalOutput").ap()

    with tile.TileContext(nc) as tc:
        with (
            tc.tile_pool(name="const", bufs=1) as const_pool,
            tc.tile_pool(name="oh", bufs=3) as oh_pool,
            tc.tile_pool(name="pgA", bufs=3, space="PSUM") as pgA_pool,
            tc.tile_pool(name="pgB", bufs=3, space="PSUM") as pgB_pool,
            tc.tile_pool(name="plog", bufs=1, space="PSUM") as plog_pool,
            tc.tile_pool(name="prsp", bufs=1, space="PSUM") as prsp_pool,
            tc.tile_pool(name="work", bufs=4) as work_pool,
            tc.tile_pool(name="stack", bufs=2) as stack_pool,
            tc.tile_pool(name="acc", bufs=2) as acc_pool,
        ):
            w_enc_hh = const_pool.tile([128, 8192], fp8, tag="wehh")
            w_enc_ih = const_pool.tile([128, 2048], fp8, tag="weih")
            w_dec_hh = const_pool.tile([128, 8192], fp8, tag="wdhh")
            w_dec_ih = const_pool.tile([128, 2048], fp8, tag="wdih")
            w_projt = const_pool.tile([128, 512], bf16, tag="wpt")
            w_projb = const_pool.tile([128, 1], f32, tag="wpb")
            ones_col = const_pool.tile([128, 1], bf16, tag="ones")
            sstA = const_pool.tile([128, 64], f32, tag="sstA")
            sstB = const_pool.tile([128, 64], f32, tag="sstB")

            nc.sync.dma_start(w_enc_hh[:], d_enc_whh)
            nc.sync.dma_start(w_enc_ih[:], d_enc_wih)
            nc.sync.dma_start(w_dec_hh[:], d_dec_whh)
            nc.sync.dma_start(w_dec_ih[:], d_dec_wih)
            nc.sync.dma_start(w_projt[:], d_projt)
            nc.sync.dma_start(w_projb[:], d_projb)
            nc.vector.memset(ones_col[:], 1.0)
            nc.vector.memset(sstA[:], 0.0)
            nc.vector.memset(sstB[:], 0.0)
            # hT rings: per sample group, per k-half (k01 / k23) so the
            # pk0 recurrent matmuls only wait on the first half of hT.
            # Slot v holds hT of step u with u%DG == v, col k*16+j (2 k each).
            ringA0 = const_pool.tile([128, DG * 32], fp8, tag="ringA0")
            ringA1 = const_pool.tile([128, DG * 32], fp8, tag="ringA1")
            ringB0 = const_pool.tile([128, DG * 32], fp8, tag="ringB0")
            ringB1 = const_pool.tile([128, DG * 32], fp8, tag="ringB1")
            for r in (ringA0, ringA1, ringB0, ringB1):
                nc.vector.memset(r[:], 0.0)
            rings = ((ringA0, ringA1), (ringB0, ringB1))
            ssts = (sstA, sstB)

            def inproj(w_ih, psq, ohq, t, q):
                """Gate-PSUM init, one group: psq col = m*16 + j."""
                for m in range(16):
                    nc.tensor.matmul(
                        psq[:, m * 16:(m + 1) * 16],
                        w_ih[:, m * 128:(m + 1) * 128],
                        ohq[:, t * BS + q * 16: t * BS + q * 16 + 16],
                        start=True, stop=False, skip_group_check=True)

            def rec_mms(w_hh, psq, q, pv):
                """DoubleRow K=256 recurrent matmuls for one sample group.
                pk0 first across all m (it only needs the k01 ring half)."""
                for pk in range(2):
                    hprev = rings[q][pk][:, pv * 32:(pv + 1) * 32]
                    for m in range(16):
                        out = psq[:, m * 16:(m + 1) * 16]
                        w = w_hh[:, (pk * 16 + m) * 256:(pk * 16 + m + 1) * 256]
                        nc.tensor.matmul(
                            out,
                            w.rearrange("p (k c) -> p k c", k=2),
                            hprev.rearrange("p (k j) -> p k j", k=2),
                            start=False, stop=(pk == 1),
                            perf_mode=DR, skip_group_check=True)

            def chain(psqs, v):
                """Both groups' activation chains, stage-interleaved."""
                Ts, tc2s = [], []
                for q in range(2):
                    T = work_pool.tile([128, 256], bf16, tag=f"T{q}",
                                       name=f"T{q}")
                    nc.scalar.activation(T[:], psqs[q][:, :], AF.Tanh,
                                         scale=1.0 / WS)
                    Ts.append(T)
                for q in range(2):
                    T, sst_q = Ts[q], ssts[q]
                    a2 = work_pool.tile([128, 64], f32, tag=f"a2{q}",
                                        name=f"a2{q}")
                    nc.vector.scalar_tensor_tensor(a2[:], T[:, 128:192], 1.0,
                                                   sst_q[:], OP.add, OP.mult)
                    a1 = work_pool.tile([128, 64], f32, tag=f"a1{q}",
                                        name=f"a1{q}")
                    nc.vector.scalar_tensor_tensor(a1[:], T[:, 64:128], 1.0,
                                                   T[:, 0:64], OP.add, OP.mult)
                    nc.vector.scalar_tensor_tensor(sst_q[:], a2[:], 0.5,
                                                   a1[:], OP.mult, OP.add)
                for q in range(2):
                    tc2 = work_pool.tile([128, 64], bf16, tag=f"tc2{q}",
                                         name=f"tc2{q}")
                    nc.scalar.activation(tc2[:], ssts[q][:], AF.Tanh, scale=0.5)
                    tc2s.append(tc2)
                for q in range(2):
                    for h in range(2):
                        nc.vector.scalar_tensor_tensor(
                            rings[q][h][:, v * 32:(v + 1) * 32],
                            Ts[q][:, 192 + h * 32:192 + (h + 1) * 32],
                            1.0, tc2s[q][:, h * 32:(h + 1) * 32],
                            OP.add, OP.mult)

            def dec_tail_mm(g):
                """Batched logits matmuls for steps DG*g .. DG*g+3."""
                ps_l = plog_pool.tile([128, 128], f32, tag="psl")
                for q in range(2):
                    for k in range(4):
                        hsrc = rings[q][k // 2][:, :].rearrange(
                            "p (u k j) -> p u k j", u=DG, k=2, j=16)
                        nc.tensor.matmul(ps_l[:, q * 64:(q + 1) * 64],
                                         w_projt[:, k * 128:(k + 1) * 128],
                                         hsrc[:, :, k % 2, :],
                                         start=(k == 0), stop=(k == 3),
                                         skip_group_check=True)
                return ps_l

            def dec_tail_reduce(ps_l, ohq, gl, accum, g, i_sym):
                """exp / tgt-dot stack (bf16), ones-matmul, accum copy."""
                st = stack_pool.tile([128, 256], bf16, tag="st")
                nc.scalar.activation(st[:, 0:128], ps_l[:, :], AF.Exp,
                                     bias=w_projb[:, 0:1], scale=1.0)
                ohv = ohq[:, :].rearrange("p (t r) -> p t r", r=32)
                for q in range(2):
                    nc.vector.tensor_mul(
                        st[:, 128 + q * 64:128 + (q + 1) * 64]
                        .rearrange("p (v j) -> p v j", v=DG),
                        ps_l[:, q * 64:(q + 1) * 64]
                        .rearrange("p (v j) -> p v j", v=DG),
                        ohv[:, gl * DG:(gl + 1) * DG, q * 16:(q + 1) * 16])
                rp = prsp_pool.tile([1, 256], f32, tag="rp")
                nc.tensor.matmul(rp[:, :], ones_col[:, 0:1], st[:, :],
                                 start=True, stop=True)
                nc.vector.tensor_copy(accum[:, g * 256:(g + 1) * 256],
                                      rp[:, :])

            def body(i, w_hh, w_ih, d_oh, dec):
                ohq = oh_pool.tile([128, U * BS], fp8, tag="oh")
                nc.sync.dma_start(ohq[:], d_oh[:, bass.ts(i, U * BS)])
                accum = None
                if dec:
                    accum = acc_pool.tile([1, (U // DG) * 256], f32, tag="accum")
                psA = pgA_pool.tile([128, 256], f32, tag="psA")
                psB = pgB_pool.tile([128, 256], f32, tag="psB")
                inproj(w_ih, psA, ohq, 0, 0)
                inproj(w_ih, psB, ohq, 0, 1)
                pend_mm = None
                pend_red = None
                for u in range(U):
                    g, v = u // DG, u % DG
                    pv = (u - 1) % DG
                    psqs = (psA, psB)
                    rec_mms(w_hh, psA, 0, pv)
                    rec_mms(w_hh, psB, 1, pv)
                    # proj matmuls for the previous dec group run here: after
                    # this step's recurrent matmuls (so they don't delay them
                    # in the PE queue) but before this step's hT overwrites
                    # ring slot 0.
                    if pend_mm is not None:
                        pend_red = (dec_tail_mm(pend_mm), ohq, pend_mm,
                                    accum, pend_mm, i)
                        pend_mm = None
                    if u + 1 < U:
                        psA_n = pgA_pool.tile([128, 256], f32, tag="psA")
                        psB_n = pgB_pool.tile([128, 256], f32, tag="psB")
                        inproj(w_ih, psA_n, ohq, u + 1, 0)
                        inproj(w_ih, psB_n, ohq, u + 1, 1)
                    chain(psqs, v)
                    if pend_red is not None:
                        dec_tail_reduce(*pend_red)
                        pend_red = None
                    if dec and v == DG - 1:
                        pend_mm = g
                    if u + 1 < U:
                        psA, psB = psA_n, psB_n
                if pend_mm is not None:
                    dec_tail_reduce(dec_tail_mm(pend_mm), ohq, pend_mm,
                                    accum, pend_mm, i)
                if dec:
                    nc.sync.dma_start(d_res[:, bass.ts(i, (U // DG) * 256)],
                                      accum[:])

            if unrolled:
                for i in range(n_iters):
                    body(i, w_enc_hh, w_enc_ih, d_enc_oh, False)
                for i in range(n_iters):
                    body(i, w_dec_hh, w_dec_ih, d_dec_oh, True)
            else:
                with tc.For_i(0, n_iters, 1, hint_engines=(PE,), name="enc") as i:
                    body(i, w_enc_hh, w_enc_ih, d_enc_oh, False)
                with tc.For_i(0, n_iters, 1, hint_engines=(PE,), name="dec") as i:
                    body(i, w_dec_hh, w_dec_ih, d_dec_oh, True)

    nc.compile()
    return nc


def _run(inputs, n_steps=S, trace=False):
    from concourse import bass_utils

    key = n_steps
    if key not in _CACHE:
        _CACHE[key] = _build_module(n_steps)
    nc = _CACHE[key]

    enc_wih, enc_whh = _prep_weights(inputs["enc_W_ih"], inputs["enc_W_hh"],
                                     inputs["enc_b_ih"], inputs["enc_b_hh"])
    dec_wih, dec_whh = _prep_weights(inputs["dec_W_ih"], inputs["dec_W_hh"],
                                     inputs["dec_b_ih"], inputs["dec_b_hh"])
    import ml_dtypes
    projW = 0.5 * np.asarray(inputs["proj_W"], np.float64)  # [128, 512]
    projt = (np.ascontiguousarray(projW.T).reshape(4, 128, 128)
             .transpose(1, 0, 2).reshape(128, 512).astype(ml_dtypes.bfloat16))
    projb = np.ascontiguousarray(
        np.asarray(inputs["proj_b"], np.float32).reshape(128, 1))

    C_idx = np.asarray(inputs["C_idx"])[:, :n_steps]
    E = np.asarray(inputs["E"])
    Etgt = E[:, :n_steps]

    in_maps = []
    for c in range(NCORES):
        sl = slice(c * BS, (c + 1) * BS)
        in_maps.append({
            "enc_whh": enc_whh, "enc_wih": enc_wih,
            "dec_whh": dec_whh, "dec_wih": dec_wih,
            "projt": np.ascontiguousarray(projt), "projb": projb,
            "enc_oh": _onehot_stream(C_idx[sl]),
            "dec_oh": _onehot_stream(Etgt[sl]),
        })

    res = bass_utils.run_bass_kernel_spmd(
        nc, in_maps, core_ids=list(range(NCORES)), trace=trace,
        trace_cores=[0] if trace else None)

    # ---- host-side loss assembly (float64) ----
    proj_b = np.asarray(inputs["proj_b"], np.float64)
    nll = np.empty((B, n_steps), np.float64)
    for c in range(NCORES):
        r = np.asarray(res.results[c]["res"], np.float64).reshape(
            n_steps // DG, 2, 2, DG, 16)       # [g, {sumexp,tgtdot}, q, v, j]
        r = r.transpose(0, 1, 3, 2, 4)         # [g, s, v, q, j]
        sumexp = r[:, 0].reshape(n_steps, BS)  # [u, sample 16q+j]
        tgtdot = r[:, 1].reshape(n_steps, BS)
        tgt = Etgt[c * BS:(c + 1) * BS]            # [j, u]
        bias_t = proj_b[tgt]                       # [j, u]
        nll[c * BS:(c + 1) * BS] = (np.log(sumexp).T
                                    - (tgtdot.T + bias_t))
    mask = (Etgt != 0).astype(np.float64)          # [B, u]
    num = (nll * mask).sum(axis=0)
    cnt = mask.sum(axis=0)
    step_loss = np.where(cnt > 0, num / np.maximum(cnt, 1.0), 0.0)
    total = np.float32(step_loss.sum())
    return total, res


def kernel(**inputs) -> np.ndarray:
    total, _ = _run(inputs, n_steps=S,
                    trace=bool(int(os.environ.get("LSTM_TRACE", "0"))))
    return np.float32(total)


# revision 7
# speedup vs baseline: 1.1077x; 1.0249x over previous
"""AutoCompleteDecoderModel (LSTM enc-dec + CE loss) on 8 Trainium2 cores.

v2 strategy (B=256, S=512, H=512, V=128; 8 cores x 32 samples):
 - gates.T [2048, 32] per step in PSUM; gate m-tile order [g|i|f|o]; bank A
   holds m 0..7 (g,i), bank B m 8..15 (f,o).
 - Flights of 2 steps share one PSUM bank pair: the input projection
   (one-hot, bias folded) for both steps of a flight is ONE matmul per
   m-tile (N=64), cutting inproj matmul count 2x.
 - Recurrent matmuls use fp8e4 weights in DoubleRow mode: K=256 per
   instruction -> 32 matmuls/step instead of 64.  h state (hT=2h) is
   stored fp8e4; weights are scaled x64 (and i/f/o rows by an extra 0.5 so
   all four gates use a single tanh scale), unscaled in the ACT pre-scale.
 - Samples split into two anti-phase groups of 16: each group's activation
   chain (1 tanh [128,256], 3 STT, tanh(c), STT) overlaps the other group's
   matmuls, hiding the chain latency behind the recurrence of the peer.
 - Decoder tail batched over groups of 4 steps: logits.T via 4 matmuls
   (N=128) reading an hT ring, one exp [128,128], one tgt-dot mul, one
   ones-matmul reduce [1,256], one copy into the result accumulator.
 - Host: nll = ln(sumexp) - (tgtdot + proj_b[tgt]), masked mean, sum.
"""

import os
import sys

import numpy as np

if "/opt/trn_rl_repo" not in sys.path:
    sys.path.insert(0, "/opt/trn_rl_repo")

B, S, H, V = 256, 512, 512, 128
NCORES = 8
BS = B // NCORES   # 32 samples per core
U = 64             # steps per hw-loop iteration
FS = 2             # steps per PSUM flight
DG = 4             # steps per decoder tail group
WS = 64.0          # fp8 weight scale

_CACHE = {}

_PERM = None


def _perm():
    global _PERM
    if _PERM is None:
        _PERM = np.concatenate([
            np.arange(1024, 1536),  # g
            np.arange(0, 512),      # i
            np.arange(512, 1024),   # f
            np.arange(1536, 2048),  # o
        ])
    return _PERM


def _prep_weights(W_ih, W_hh, b_ih, b_hh):
    """Fold biases into W_ih, fold the hT=2h and single-tanh-scale factors,
    scale x64, quantize fp8e4, and pack for the kernel layouts."""
    import ml_dtypes

    fp8 = ml_dtypes.float8_e4m3
    perm = _perm()
    Wi = (np.asarray(W_ih, np.float64) + (np.asarray(b_ih, np.float64)
          + np.asarray(b_hh, np.float64))[:, None])[perm]  # [2048, 128]
    Wh = 0.5 * np.asarray(W_hh, np.float64)[perm]          # [2048, 512]
    Wi[512:] *= 0.5  # i,f,o rows: single tanh scale (tanh(z/2))
    Wh[512:] *= 0.5
    Wi *= WS
    Wh *= WS
    # input proj lhsT tiles: wih_t [V=128, 2048], m-tile m at cols m*128
    wih_t = np.ascontiguousarray(Wi.T).astype(fp8)
    # recurrent DoubleRow pairs: whh_dr [128, 8192], block (pk, m) at col
    # (pk*16+m)*256, within block [kk=2, c=128]; k-tile = 2*pk+kk
    Wt = np.ascontiguousarray(Wh.T)                        # [512, 2048]
    whh_dr = (Wt.reshape(2, 2, 128, 16, 128)               # [pk,kk,p,m,c]
              .transpose(2, 0, 3, 1, 4)                    # [p,pk,m,kk,c]
              .reshape(128, 8192).astype(fp8))
    return np.ascontiguousarray(wih_t), np.ascontiguousarray(whh_dr)


def _onehot_stream(idx):
    """idx [BS, S] int -> [128, S*32] fp8e4, col t*32+j = (idx[j,t]==v)."""
    import ml_dtypes
    oh = (np.arange(V, dtype=np.int32)[:, None, None]
          == np.asarray(idx, np.int32).T[None, :, :])  # [V, S, BS]
    return np.ascontiguousarray(
        oh.reshape(V, -1).astype(ml_dtypes.float8_e4m3))


def _build_module(n_steps, unrolled=False):
    _ABL_TAIL = bool(int(os.environ.get("ABL_TAIL", "0")))
    import concourse.bacc as bacc
    import concourse.bass as bass
    import concourse.mybir as mybir
    import concourse.tile as tile

    f32 = mybir.dt.float32
    bf16 = mybir.dt.bfloat16
    fp8 = mybir.dt.float8e4
    AF = mybir.ActivationFunctionType
    OP = mybir.AluOpType
    PE = mybir.EngineType.PE
    DR = mybir.MatmulPerfMode.DoubleRow

    assert n_steps % U == 0
    n_iters = n_steps // U

    nc = bacc.Bacc("TRN2", target_bir_lowering=False, debug=False,
                   num_devices=NCORES)

    d_enc_whh = nc.dram_tensor("enc_whh", [128, 8192], fp8, kind="ExternalInput").ap()
    d_enc_wih = nc.dram_tensor("enc_wih", [128, 2048], fp8, kind="ExternalInput").ap()
    d_dec_whh = nc.dram_tensor("dec_whh", [128, 8192], fp8, kind="ExternalInput").ap()
    d_dec_wih = nc.dram_tensor("dec_wih", [128, 2048], fp8, kind="ExternalInput").ap()
    d_projt = nc.dram_tensor("projt", [128, 512], bf16, kind="ExternalInput").ap()
    d_projb = nc.dram_tensor("projb", [128, 1], f32, kind="ExternalInput").ap()
    d_enc_oh = nc.dram_tensor("enc_oh", [128, n_steps * BS], fp8, kind="ExternalInput").ap()
    d_dec_oh = nc.dram_tensor("dec_oh", [128, n_steps * BS], fp8, kind="ExternalInput").ap()
    d_res = nc.dram_tensor("res", [1, (n_steps // DG) * 256], f32, kind="ExternalOutput").ap()

    with tile.TileContext(nc) as tc:
        with (
            tc.tile_pool(name="const", bufs=1) as const_pool,
            tc.tile_pool(name="oh", bufs=3) as oh_pool,
            tc.tile_pool(name="pgA", bufs=3, space="PSUM") as pgA_pool,
            tc.tile_pool(name="pgB", bufs=3, space="PSUM") as pgB_pool,
            tc.tile_pool(name="plog", bufs=1, space="PSUM") as plog_pool,
            tc.tile_pool(name="prsp", bufs=1, space="PSUM") as prsp_pool,
            tc.tile_pool(name="work", bufs=4) as work_pool,
            tc.tile_pool(name="stack", bufs=2) as stack_pool,
            tc.tile_pool(name="acc", bufs=2) as acc_pool,
        ):
            w_enc_hh = const_pool.tile([128, 8192], fp8, tag="wehh")
            w_enc_ih = const_pool.tile([128, 2048], fp8, tag="weih")
            w_dec_hh = const_pool.tile([128, 8192], fp8, tag="wdhh")
            w_dec_ih = const_pool.tile([128, 2048], fp8, tag="wdih")
            w_projt = const_pool.tile([128, 512], bf16, tag="wpt")
            w_projb = const_pool.tile([128, 1], f32, tag="wpb")
            ones_col = const_pool.tile([128, 1], bf16, tag="ones")
            sstA = const_pool.tile([128, 64], f32, tag="sstA")
            sstB = const_pool.tile([128, 64], f32, tag="sstB")

            # Encoder weights first (the first steps need wih then whh);
            # decoder weights + projection load after the encoder loop is
            # issued, overlapping the 512 encoder steps.
            nc.sync.dma_start(w_enc_ih[:], d_enc_wih)
            nc.sync.dma_start(w_enc_hh[:], d_enc_whh)
            nc.vector.memset(ones_col[:], 1.0)
            nc.vector.memset(sstA[:], 0.0)
            nc.vector.memset(sstB[:], 0.0)
            # hT rings: per sample group, per k-half (k01 / k23) so the
            # pk0 recurrent matmuls only wait on the first half of hT.
            # Slot v holds hT of step u with u%DG == v, col k*16+j (2 k each).
            ringA0 = const_pool.tile([128, DG * 32], fp8, tag="ringA0")
            ringA1 = const_pool.tile([128, DG * 32], fp8, tag="ringA1")
            ringB0 = const_pool.tile([128, DG * 32], fp8, tag="ringB0")
            ringB1 = const_pool.tile([128, DG * 32], fp8, tag="ringB1")
            for r in (ringA0, ringA1, ringB0, ringB1):
                nc.vector.memset(r[:], 0.0)
            rings = ((ringA0, ringA1), (ringB0, ringB1))
            ssts = (sstA, sstB)

            def inproj(w_ih, psq, ohq, t, q):
                """Gate-PSUM init, one group: psq col = m*16 + j."""
                for m in range(16):
                    nc.tensor.matmul(
                        psq[:, m * 16:(m + 1) * 16],
                        w_ih[:, m * 128:(m + 1) * 128],
                        ohq[:, t * BS + q * 16: t * BS + q * 16 + 16],
                        start=True, stop=False, skip_group_check=True)

            def rec_mms(w_hh, psq, q, pv):
                """DoubleRow K=256 recurrent matmuls for one sample group.
                pk0 first across all m (it only needs the k01 ring half)."""
                for pk in range(2):
                    hprev = rings[q][pk][:, pv * 32:(pv + 1) * 32]
                    for m in range(16):
                        out = psq[:, m * 16:(m + 1) * 16]
                        w = w_hh[:, (pk * 16 + m) * 256:(pk * 16 + m + 1) * 256]
                        nc.tensor.matmul(
                            out,
                            w.rearrange("p (k c) -> p k c", k=2),
                            hprev.rearrange("p (k j) -> p k j", k=2),
                            start=False, stop=(pk == 1),
                            perf_mode=DR, skip_group_check=True)

            def chain(psqs, v):
                """Both groups' activation chains, stage-interleaved."""
                Ts, tc2s = [], []
                for q in range(2):
                    T = work_pool.tile([128, 256], bf16, tag=f"T{q}",
                                       name=f"T{q}")
                    nc.scalar.activation(T[:], psqs[q][:, :], AF.Tanh,
                                         scale=1.0 / WS)
                    Ts.append(T)
                for q in range(2):
                    T, sst_q = Ts[q], ssts[q]
                    a2 = work_pool.tile([128, 64], f32, tag=f"a2{q}",
                                        name=f"a2{q}")
                    nc.vector.scalar_tensor_tensor(a2[:], T[:, 128:192], 1.0,
                                                   sst_q[:], OP.add, OP.mult)
                    a1 = work_pool.tile([128, 64], f32, tag=f"a1{q}",
                                        name=f"a1{q}")
                    nc.vector.scalar_tensor_tensor(a1[:], T[:, 64:128], 1.0,
                                                   T[:, 0:64], OP.add, OP.mult)
                    nc.vector.scalar_tensor_tensor(sst_q[:], a2[:], 0.5,
                                                   a1[:], OP.mult, OP.add)
                for q in range(2):
                    tc2 = work_pool.tile([128, 64], bf16, tag=f"tc2{q}",
                                         name=f"tc2{q}")
                    nc.scalar.activation(tc2[:], ssts[q][:], AF.Tanh, scale=0.5)
                    tc2s.append(tc2)
                for q in range(2):
                    for h in range(2):
                        nc.vector.scalar_tensor_tensor(
                            rings[q][h][:, v * 32:(v + 1) * 32],
                            Ts[q][:, 192 + h * 32:192 + (h + 1) * 32],
                            1.0, tc2s[q][:, h * 32:(h + 1) * 32],
                            OP.add, OP.mult)

            def dec_tail_mm(g):
                """Batched logits matmuls for steps DG*g .. DG*g+3."""
                ps_l = plog_pool.tile([128, 128], f32, tag="psl")
                for q in range(2):
                    for k in range(4):
                        hsrc = rings[q][k // 2][:, :].rearrange(
                            "p (u k j) -> p u k j", u=DG, k=2, j=16)
                        nc.tensor.matmul(ps_l[:, q * 64:(q + 1) * 64],
                                         w_projt[:, k * 128:(k + 1) * 128],
                                         hsrc[:, :, k % 2, :],
                                         start=(k == 0), stop=(k == 3),
                                         skip_group_check=True)
                return ps_l

            def dec_tail_reduce(ps_l, ohq, gl, accum, g, i_sym):
                """exp / tgt-dot stack (bf16), ones-matmul, accum copy."""
                st = stack_pool.tile([128, 256], bf16, tag="st")
                nc.scalar.activation(st[:, 0:128], ps_l[:, :], AF.Exp,
                                     bias=w_projb[:, 0:1], scale=1.0)
                ohv = ohq[:, :].rearrange("p (t r) -> p t r", r=32)
                for q in range(2):
                    nc.vector.tensor_mul(
                        st[:, 128 + q * 64:128 + (q + 1) * 64]
                        .rearrange("p (v j) -> p v j", v=DG),
                        ps_l[:, q * 64:(q + 1) * 64]
                        .rearrange("p (v j) -> p v j", v=DG),
                        ohv[:, gl * DG:(gl + 1) * DG, q * 16:(q + 1) * 16])
                rp = prsp_pool.tile([1, 256], f32, tag="rp")
                nc.tensor.matmul(rp[:, :], ones_col[:, 0:1], st[:, :],
                                 start=True, stop=True)
                nc.vector.tensor_copy(accum[:, g * 256:(g + 1) * 256],
                                      rp[:, :])

            def body(i, w_hh, w_ih, d_oh, dec):
                ohq = oh_pool.tile([128, U * BS], fp8, tag="oh")
                nc.sync.dma_start(ohq[:], d_oh[:, bass.ts(i, U * BS)])
                accum = None
                if dec:
                    accum = acc_pool.tile([1, (U // DG) * 256], f32, tag="accum")
                psA = pgA_pool.tile([128, 256], f32, tag="psA")
                psB = pgB_pool.tile([128, 256], f32, tag="psB")
                inproj(w_ih, psA, ohq, 0, 0)
                inproj(w_ih, psB, ohq, 0, 1)
                pend_mm = None
                pend_red = None
                for u in range(U):
                    g, v = u // DG, u % DG
                    pv = (u - 1) % DG
                    psqs = (psA, psB)
                    rec_mms(w_hh, psA, 0, pv)
                    rec_mms(w_hh, psB, 1, pv)
                    # proj matmuls for the previous dec group run here: after
                    # this step's recurrent matmuls (so they don't delay them
                    # in the PE queue) but before this step's hT overwrites
                    # ring slot 0.
                    if pend_mm is not None and not _ABL_TAIL:
                        pend_red = (dec_tail_mm(pend_mm), ohq, pend_mm,
                                    accum, pend_mm, i)
                        pend_mm = None
                    pend_mm = None if _ABL_TAIL else pend_mm
                    if u + 1 < U:
                        psA_n = pgA_pool.tile([128, 256], f32, tag="psA")
                        psB_n = pgB_pool.tile([128, 256], f32, tag="psB")
                        inproj(w_ih, psA_n, ohq, u + 1, 0)
                        inproj(w_ih, psB_n, ohq, u + 1, 1)
                    chain(psqs, v)
                    if pend_red is not None:
                        dec_tail_reduce(*pend_red)
                        pend_red = None
                    if dec and v == DG - 1:
                        pend_mm = g
                    if u + 1 < U:
                        psA, psB = psA_n, psB_n
                if pend_mm is not None:
                    dec_tail_reduce(dec_tail_mm(pend_mm), ohq, pend_mm,
                                    accum, pend_mm, i)
                if dec:
                    nc.sync.dma_start(d_res[:, bass.ts(i, (U // DG) * 256)],
                                      accum[:])

            if unrolled:
                for i in range(n_iters):
                    body(i, w_enc_hh, w_enc_ih, d_enc_oh, False)
                nc.sync.dma_start(w_dec_ih[:], d_dec_wih)
                nc.sync.dma_start(w_dec_hh[:], d_dec_whh)
                nc.sync.dma_start(w_projt[:], d_projt)
                nc.sync.dma_start(w_projb[:], d_projb)
                for i in range(n_iters):
                    body(i, w_dec_hh, w_dec_ih, d_dec_oh, True)
            else:
                with tc.For_i(0, n_iters, 1, hint_engines=(PE,), name="enc") as i:
                    body(i, w_enc_hh, w_enc_ih, d_enc_oh, False)
                nc.sync.dma_start(w_dec_ih[:], d_dec_wih)
                nc.sync.dma_start(w_dec_hh[:], d_dec_whh)
                nc.sync.dma_start(w_projt[:], d_projt)
                nc.sync.dma_start(w_projb[:], d_projb)
                with tc.For_i(0, n_iters, 1, hint_engines=(PE,), name="dec") as i:
                    body(i, w_dec_hh, w_dec_ih, d_dec_oh, True)

    nc.compile()
    return nc


def _run(inputs, n_steps=S, trace=False):
    from concourse import bass_utils

    key = n_steps
    if key not in _CACHE:
        _CACHE[key] = _build_module(n_steps)
    nc = _CACHE[key]

    enc_wih, enc_whh = _prep_weights(inputs["enc_W_ih"], inputs["enc_W_hh"],
                                     inputs["enc_b_ih"], inputs["enc_b_hh"])
    dec_wih, dec_whh = _prep_weights(inputs["dec_W_ih"], inputs["dec_W_hh"],
                                     inputs["dec_b_ih"], inputs["dec_b_hh"])
    import ml_dtypes
    projW = 0.5 * np.asarray(inputs["proj_W"], np.float64)  # [128, 512]
    projt = (np.ascontiguousarray(projW.T).reshape(4, 128, 128)
             .transpose(1, 0, 2).reshape(128, 512).astype(ml_dtypes.bfloat16))
    projb = np.ascontiguousarray(
        np.asarray(inputs["proj_b"], np.float32).reshape(128, 1))

    C_idx = np.asarray(inputs["C_idx"])[:, :n_steps]
    E = np.asarray(inputs["E"])
    Etgt = E[:, :n_steps]

    in_maps = []
    for c in range(NCORES):
        sl = slice(c * BS, (c + 1) * BS)
        in_maps.append({
            "enc_whh": enc_whh, "enc_wih": enc_wih,
            "dec_whh": dec_whh, "dec_wih": dec_wih,
            "projt": np.ascontiguousarray(projt), "projb": projb,
            "enc_oh": _onehot_stream(C_idx[sl]),
            "dec_oh": _onehot_stream(Etgt[sl]),
        })

    res = bass_utils.run_bass_kernel_spmd(
        nc, in_maps, core_ids=list(range(NCORES)), trace=trace,
        trace_cores=[0] if trace else None)

    # ---- host-side loss assembly (float64) ----
    proj_b = np.asarray(inputs["proj_b"], np.float64)
    nll = np.empty((B, n_steps), np.float64)
    for c in range(NCORES):
        r = np.asarray(res.results[c]["res"], np.float64).reshape(
            n_steps // DG, 2, 2, DG, 16)       # [g, {sumexp,tgtdot}, q, v, j]
        r = r.transpose(0, 1, 3, 2, 4)         # [g, s, v, q, j]
        sumexp = r[:, 0].reshape(n_steps, BS)  # [u, sample 16q+j]
        tgtdot = r[:, 1].reshape(n_steps, BS)
        tgt = Etgt[c * BS:(c + 1) * BS]            # [j, u]
        bias_t = proj_b[tgt]                       # [j, u]
        nll[c * BS:(c + 1) * BS] = (np.log(sumexp).T
                                    - (tgtdot.T + bias_t))
    mask = (Etgt != 0).astype(np.float64)          # [B, u]
    num = (nll * mask).sum(axis=0)
    cnt = mask.sum(axis=0)
    step_loss = np.where(cnt > 0, num / np.maximum(cnt, 1.0), 0.0)
    total = np.float32(step_loss.sum())
    return total, res


def kernel(**inputs) -> np.ndarray:
    total, _ = _run(inputs, n_steps=S,
                    trace=bool(int(os.environ.get("LSTM_TRACE", "0"))))
    return np.float32(total)
